# revision 1
# baseline (speedup 1.0000x reference)
"""GCN+MLP Trainium2 kernel: 8-core SPMD, NKI sparse aggregation + XLA dense.

Model (see harness reference): embed -> 2x ChebConv(K=2) -> mean-pool -> MLP
-> broadcast. N=65536 nodes, 1M random edges (uniform), EMB=128.

Distribution: nodes dst-sharded 8192/core (= 4 complete batches, so the mean
pool is local). Edges are routed on host to the core owning their dst and
sorted by dst. Aggregation per core, per dst-group of 128 nodes:
  - indirect row DMA (vector DGE) gathers 128 source rows per chunk,
  - a one-hot matrix (iota==dst_local)*wgt built on the vector engine,
  - TensorEngine matmul accumulates chunk messages into PSUM: txT += G.T @ OH.
ChebConv linear algebra identities move all per-edge scaling into the one-hot
(wgt = -dinv[src]*dinv[dst]) and the embed matmul *after* aggregation for
conv0 (aggregate raw x, then multiply by embed_W). The h1 table is
all-gathered (bf16, 2MB/core) between conv layers; dense matmuls, pooling and
the MLP run in XLA.
"""

import numpy as np

import jax
import jax.numpy as jnp
from jax.sharding import Mesh, PartitionSpec as P
from jax.experimental.shard_map import shard_map

import neuronxcc.nki as nki
import neuronxcc.nki.isa as nisa
import neuronxcc.nki.language as nl

B, E, D = 32, 2048, 64
EMB, HID, PRED, NPRED = 128, 64, 3, 12
N = B * E
NCORES = 8
NPC = N // NCORES          # 8192 nodes/core
BPC = B // NCORES          # 4 batches/core

NGRP = 64                  # dst groups of 128 per core
DGRP = NPC // NGRP         # 128
NCH = 18                   # chunks of 128 edge slots per group (mean 16)
NCHK = NGRP * NCH          # 1152 chunks per core per layer

_CACHE = {}


def _agg_kernel_factory(PF, bf16):
    """Aggregation kernel: table [N, PF] -> txT [PF, NPC] f32."""
    mdt = nl.bfloat16 if bf16 else nl.float32

    @nki.jit
    def agg_kernel(table, idxs, dstw, iota):
        # table: [N, PF] (bf16|f32) node-major gather source in HBM
        # idxs:  [128, NCHK] int32   (chunk c -> column c)
        # dstw:  [2, 128, NCHK] (f32) [dst_local | wgt]
        # iota:  [128, DGRP] (mdt)  iota[e, d] = d
        out = nl.ndarray((PF, NPC), dtype=nl.float32, buffer=nl.shared_hbm)
        ix = nl.load(idxs)                          # [128, NCHK]
        dl = nl.load(dstw[0])                       # [128, NCHK] f32
        wg = nl.load(dstw[1])                       # [128, NCHK] f32
        io_t = nl.load(iota)                        # [128, DGRP]
        i_p, _ = nl.mgrid[:128, :1]
        _, i_f = nl.mgrid[:128, :PF]
        for g in range(NGRP):
            ps0 = nl.zeros((PF, DGRP), dtype=nl.float32, buffer=nl.psum)
            ps1 = nl.zeros((PF, DGRP), dtype=nl.float32, buffer=nl.psum)
            tmp = nl.ndarray((NCH, nl.par_dim(128), PF), dtype=mdt,
                             buffer=nl.sbuf)
            for ch in range(NCH):
                c = g * NCH + ch
                nisa.dma_copy(dst=tmp[ch], src=table[ix[i_p, c], i_f])
            for ch in range(NCH):
                c = g * NCH + ch
                oh = nisa.tensor_scalar(
                    io_t, op0=nl.equal, operand0=dl[:, c:c + 1],
                    op1=nl.multiply, operand1=wg[:, c:c + 1],
                    dtype=mdt)
                if ch % 2 == 0:
                    ps0 += nisa.nc_matmul(tmp[ch], oh)
                else:
                    ps1 += nisa.nc_matmul(tmp[ch], oh)
            sl = slice(g * DGRP, (g + 1) * DGRP)
            out_sb = nl.add(ps0, ps1)
            nl.store(out[:, sl], out_sb)
        return out

    return agg_kernel


def _prep_edges(src, dst, dinv):
    """Route edges to cores, sort by dst, chunk per 128-dst groups."""
    idx_all = np.zeros((NCORES, 128, NCHK), np.int32)
    dstw_all = np.zeros((NCORES, 2, 128, NCHK), np.float32)
    wgt = (-dinv[src] * dinv[dst]).astype(np.float32)
    core = dst >> 13
    order = np.argsort(core * np.int64(N) + dst, kind="stable")
    src_s, dst_s, wgt_s = src[order], dst[order], wgt[order]
    cstart = np.searchsorted(core[order], np.arange(NCORES + 1))
    for c in range(NCORES):
        s_c = src_s[cstart[c]:cstart[c + 1]]
        d_c = dst_s[cstart[c]:cstart[c + 1]] - c * NPC
        w_c = wgt_s[cstart[c]:cstart[c + 1]]
        grp = d_c >> 7
        gstart = np.searchsorted(grp, np.arange(NGRP + 1))
        for g in range(NGRP):
            lo, hi = gstart[g], gstart[g + 1]
            n = hi - lo
            if n > NCH * 128:
                raise RuntimeError(f"group overflow {n} > {NCH * 128}")
            # slot j of group -> chunk j//128, partition j%128
            sl = np.arange(n)
            ch, pt = g * NCH + sl // 128, sl % 128
            idx_all[c, pt, ch] = s_c[lo:hi]
            dstw_all[c, 0, pt, ch] = (d_c[lo:hi] & 127).astype(np.float32)
            dstw_all[c, 1, pt, ch] = w_c[lo:hi]
            # padding slots keep wgt=0 -> zero contribution
    return idx_all, dstw_all


def kernel(**inputs):
    x = np.asarray(inputs["x"], np.float32)
    edge_index = np.asarray(inputs["edge_index"])
    W_e = np.asarray(inputs["embed_W"], np.float32)
    b_e = np.asarray(inputs["embed_b"], np.float32)
    W00 = np.asarray(inputs["conv0_W0"], np.float32)
    W01 = np.asarray(inputs["conv0_W1"], np.float32)
    b0 = np.asarray(inputs["conv0_b"], np.float32)
    W10 = np.asarray(inputs["conv1_W0"], np.float32)
    W11 = np.asarray(inputs["conv1_W1"], np.float32)
    b1 = np.asarray(inputs["conv1_b"], np.float32)
    mW1 = np.asarray(inputs["mlp_W1"], np.float32)
    mb1 = np.asarray(inputs["mlp_b1"], np.float32)
    mW2 = np.asarray(inputs["mlp_W2"], np.float32)
    mb2 = np.asarray(inputs["mlp_b2"], np.float32)

    src = np.asarray(edge_index[0], np.int64).astype(np.int32)
    dst = np.asarray(edge_index[1], np.int64).astype(np.int32)
    deg = np.bincount(src, minlength=N).astype(np.float32)
    dinv = np.where(deg > 0, 1.0 / np.sqrt(np.maximum(deg, 1e-12)), 0.0).astype(np.float32)

    idx_all, dstw_all = _prep_edges(src, dst, dinv)

    # rank-1 term for the embed bias routed through aggregation:
    # tx0 = Agg(x) @ We + sw * b_e, with sw[d] = sum of in-edge wgts of d
    sw = np.zeros(N, np.float32)
    np.add.at(sw, dst, (-dinv[src] * dinv[dst]))
    sw_sh = sw.reshape(NCORES, NPC)
    xT_sh = x.reshape(NCORES, NPC, D)
    iota = np.tile(np.arange(DGRP, dtype=np.float32), (128, 1))

    agg64 = _CACHE.setdefault("agg64", _agg_kernel_factory(D, True))
    agg128 = _CACHE.setdefault("agg128", _agg_kernel_factory(EMB, True))

    devices = jax.devices()[:NCORES]
    mesh = Mesh(np.array(devices), ("x",))

    consts = dict(
        W_e=jnp.asarray(W_e), b_e=jnp.asarray(b_e),
        W00=jnp.asarray(W00), W01=jnp.asarray(W01), b0=jnp.asarray(b0),
        W10=jnp.asarray(W10), W11=jnp.asarray(W11), b1=jnp.asarray(b1),
        mW1=jnp.asarray(mW1), mb1=jnp.asarray(mb1),
        mW2=jnp.asarray(mW2), mb2=jnp.asarray(mb2),
        xtab=jnp.asarray(x.reshape(N, D), jnp.bfloat16),
        iota32=jnp.asarray(iota, jnp.float32),
        iota16=jnp.asarray(iota, jnp.bfloat16),
    )
    const_names = list(consts.keys())
    const_vals = [consts[k] for k in const_names]

    def shard_fn(idx, dstw, xc, dinvc, swc, *cv):
        cst = dict(zip(const_names, cv))
        idx0, dstw0 = idx[0], dstw[0]
        xc, dinvc, swc = xc[0], dinvc[0], swc[0]
        # conv0: aggregate raw x rows, then embed the aggregate
        aggx = agg64(cst["xtab"], idx0, dstw0, cst["iota16"])   # [64, 8192] f32
        h0T = (xc @ cst["W_e"] + cst["b_e"]).T                   # [128, 8192]
        tx0T = cst["W_e"].T @ aggx + cst["b_e"][:, None] * swc[None, :]
        h1T = jax.nn.relu(cst["W00"].T @ h0T + cst["W01"].T @ tx0T + cst["b0"][:, None])
        u1 = h1T.T.astype(jnp.bfloat16)                          # [8192, 128] node-major
        u1full = jax.lax.all_gather(u1, "x", axis=0, tiled=True)  # [65536, 128]
        agg1 = agg128(u1full, idx0, dstw0, cst["iota16"])        # [128, 8192] f32
        h2T = jax.nn.relu(cst["W10"].T @ h1T + cst["W11"].T @ agg1 + cst["b1"][:, None])
        hm = h2T.reshape(EMB, BPC, E).mean(axis=2)               # [128, 4]
        z = jax.nn.relu(cst["mW1"].T @ hm + cst["mb1"][:, None])
        o = cst["mW2"].T @ z + cst["mb2"][:, None]               # [3, 4]
        return o.T[None]                                         # [1, 4, 3]

    fn = shard_map(
        shard_fn, mesh=mesh,
        in_specs=(P("x"), P("x"), P("x"), P("x"), P("x"))
                 + tuple(P() for _ in const_vals),
        out_specs=P("x"),
        check_rep=False,
    )
    jfn = jax.jit(fn)
    out = jfn(jnp.asarray(idx_all), jnp.asarray(dstw_all),
              jnp.asarray(xT_sh), jnp.asarray(dinv_sh := dinv.reshape(NCORES, NPC)),
              jnp.asarray(sw_sh), *const_vals)
    out1 = np.asarray(out, np.float32).reshape(B, PRED)
    full = np.broadcast_to(out1[:, None, None, :], (B, NPRED, E, PRED))
    return np.ascontiguousarray(full, dtype=np.float32)


kernel._jit_holder = _CACHE



# revision 3
# speedup vs baseline: 8.3591x; 8.3591x over previous
"""GCN+MLP Trainium2 kernel: 8-core SPMD, NKI sparse aggregation + XLA dense.

Model (see harness reference): embed -> 2x ChebConv(K=2) -> mean-pool -> MLP
-> broadcast. N=65536 nodes, 1M random edges (uniform), EMB=128.

Distribution: nodes dst-sharded 8192/core (= 4 complete batches, so the mean
pool is local). Edges are routed on host to the core owning their dst and
sorted by dst. Aggregation per core, per dst-group of 128 nodes:
  - indirect row DMA (vector DGE) gathers 128 source rows per chunk,
  - a one-hot matrix (iota==dst_local)*wgt built on the vector engine,
  - TensorEngine matmul accumulates chunk messages into PSUM: txT += G.T @ OH.
ChebConv linear algebra identities move all per-edge scaling into the one-hot
(wgt = -dinv[src]*dinv[dst]) and the embed matmul *after* aggregation for
conv0 (aggregate raw x, then multiply by embed_W). The h1 table is
all-gathered (bf16, 2MB/core) between conv layers; dense matmuls, pooling and
the MLP run in XLA.
"""

import numpy as np

import jax
import jax.numpy as jnp
from jax.sharding import Mesh, PartitionSpec as P
from jax.experimental.shard_map import shard_map

import neuronxcc.nki as nki
import neuronxcc.nki.isa as nisa
import neuronxcc.nki.language as nl

B, E, D = 32, 2048, 64
EMB, HID, PRED, NPRED = 128, 64, 3, 12
N = B * E
NCORES = 8
NPC = N // NCORES          # 8192 nodes/core
BPC = B // NCORES          # 4 batches/core

NGRP = 64                  # dst groups of 128 per core
DGRP = NPC // NGRP         # 128
NCH = 18                   # chunks of 128 edge slots per group (mean 16)
NCHK = NGRP * NCH          # 1152 chunks per core per layer

_CACHE = {}


def _agg_kernel_factory(PF, bf16):
    """Aggregation kernel: table [N, PF] -> txT [PF, NPC] f32."""
    mdt = nl.bfloat16 if bf16 else nl.float32

    @nki.jit
    def agg_kernel(table, idxs, dstw, iota):
        # table: [N, PF] (bf16|f32) node-major gather source in HBM
        # idxs:  [128, NCHK] int32   (chunk c -> column c)
        # dstw:  [2, 128, NCHK] (f32) [dst_local | wgt]
        # iota:  [128, DGRP] (mdt)  iota[e, d] = d
        out = nl.ndarray((PF, NPC), dtype=nl.float32, buffer=nl.shared_hbm)
        ix = nl.load(idxs)                          # [128, NCHK]
        dl = nl.load(dstw[0])                       # [128, NCHK] f32
        wg = nl.load(dstw[1])                       # [128, NCHK] f32
        io_t = nl.load(iota)                        # [128, DGRP]
        i_p = nl.arange(128)[:, None]
        i_f = nl.arange(PF)[None, :]
        for g in range(NGRP):
            ps0 = nl.zeros((PF, DGRP), dtype=nl.float32, buffer=nl.psum)
            ps1 = nl.zeros((PF, DGRP), dtype=nl.float32, buffer=nl.psum)
            tmp = nl.ndarray((nl.par_dim(128), NCH, PF), dtype=mdt,
                             buffer=nl.sbuf)
            for ch in range(NCH):
                c = g * NCH + ch
                nisa.dma_copy(dst=tmp[:, ch], src=table[ix[i_p, c], i_f],
                              dge_mode=nisa.dge_mode.hwdge)
            for ch in range(NCH):
                c = g * NCH + ch
                oh = nisa.tensor_scalar(
                    io_t, op0=nl.equal, operand0=dl[:, c:c + 1],
                    op1=nl.multiply, operand1=wg[:, c:c + 1],
                    dtype=mdt)
                if ch % 2 == 0:
                    ps0 += nisa.nc_matmul(tmp[:, ch], oh)
                else:
                    ps1 += nisa.nc_matmul(tmp[:, ch], oh)
            sl = slice(g * DGRP, (g + 1) * DGRP)
            out_sb = nl.add(ps0, ps1)
            nl.store(out[:, sl], out_sb)
        return out

    return agg_kernel


def _prep_edges(src, dst, dinv):
    """Route edges to cores, sort by dst, chunk per 128-dst groups."""
    idx_all = np.zeros((NCORES, 128, NCHK), np.int32)
    dstw_all = np.zeros((NCORES, 2, 128, NCHK), np.float32)
    wgt = (-dinv[src] * dinv[dst]).astype(np.float32)
    core = dst >> 13
    order = np.argsort(core * np.int64(N) + dst, kind="stable")
    src_s, dst_s, wgt_s = src[order], dst[order], wgt[order]
    cstart = np.searchsorted(core[order], np.arange(NCORES + 1))
    for c in range(NCORES):
        s_c = src_s[cstart[c]:cstart[c + 1]]
        d_c = dst_s[cstart[c]:cstart[c + 1]] - c * NPC
        w_c = wgt_s[cstart[c]:cstart[c + 1]]
        grp = d_c >> 7
        gstart = np.searchsorted(grp, np.arange(NGRP + 1))
        for g in range(NGRP):
            lo, hi = gstart[g], gstart[g + 1]
            n = hi - lo
            if n > NCH * 128:
                raise RuntimeError(f"group overflow {n} > {NCH * 128}")
            # slot j of group -> chunk j//128, partition j%128
            sl = np.arange(n)
            ch, pt = g * NCH + sl // 128, sl % 128
            idx_all[c, pt, ch] = s_c[lo:hi]
            dstw_all[c, 0, pt, ch] = (d_c[lo:hi] & 127).astype(np.float32)
            dstw_all[c, 1, pt, ch] = w_c[lo:hi]
            # padding slots keep wgt=0 -> zero contribution
    return idx_all, dstw_all


def kernel(**inputs):
    x = np.asarray(inputs["x"], np.float32)
    edge_index = np.asarray(inputs["edge_index"])
    W_e = np.asarray(inputs["embed_W"], np.float32)
    b_e = np.asarray(inputs["embed_b"], np.float32)
    W00 = np.asarray(inputs["conv0_W0"], np.float32)
    W01 = np.asarray(inputs["conv0_W1"], np.float32)
    b0 = np.asarray(inputs["conv0_b"], np.float32)
    W10 = np.asarray(inputs["conv1_W0"], np.float32)
    W11 = np.asarray(inputs["conv1_W1"], np.float32)
    b1 = np.asarray(inputs["conv1_b"], np.float32)
    mW1 = np.asarray(inputs["mlp_W1"], np.float32)
    mb1 = np.asarray(inputs["mlp_b1"], np.float32)
    mW2 = np.asarray(inputs["mlp_W2"], np.float32)
    mb2 = np.asarray(inputs["mlp_b2"], np.float32)

    src = np.asarray(edge_index[0], np.int64).astype(np.int32)
    dst = np.asarray(edge_index[1], np.int64).astype(np.int32)
    deg = np.bincount(src, minlength=N).astype(np.float32)
    dinv = np.where(deg > 0, 1.0 / np.sqrt(np.maximum(deg, 1e-12)), 0.0).astype(np.float32)

    idx_all, dstw_all = _prep_edges(src, dst, dinv)

    # rank-1 term for the embed bias routed through aggregation:
    # tx0 = Agg(x) @ We + sw * b_e, with sw[d] = sum of in-edge wgts of d
    sw = np.zeros(N, np.float32)
    np.add.at(sw, dst, (-dinv[src] * dinv[dst]))
    sw_sh = sw.reshape(NCORES, NPC)
    xT_sh = x.reshape(NCORES, NPC, D)
    iota = np.tile(np.arange(DGRP, dtype=np.float32), (128, 1))

    agg64 = _CACHE.setdefault("agg64", _agg_kernel_factory(D, True))
    agg128 = _CACHE.setdefault("agg128", _agg_kernel_factory(EMB, True))

    devices = jax.devices()[:NCORES]
    mesh = Mesh(np.array(devices), ("x",))

    consts = dict(
        W_e=jnp.asarray(W_e), b_e=jnp.asarray(b_e),
        W00=jnp.asarray(W00), W01=jnp.asarray(W01), b0=jnp.asarray(b0),
        W10=jnp.asarray(W10), W11=jnp.asarray(W11), b1=jnp.asarray(b1),
        mW1=jnp.asarray(mW1), mb1=jnp.asarray(mb1),
        mW2=jnp.asarray(mW2), mb2=jnp.asarray(mb2),
        xtab=jnp.asarray(x.reshape(N, D), jnp.bfloat16),
        iota32=jnp.asarray(iota, jnp.float32),
        iota16=jnp.asarray(iota, jnp.bfloat16),
    )
    const_names = list(consts.keys())
    const_vals = [consts[k] for k in const_names]

    def shard_fn(idx, dstw, xc, dinvc, swc, *cv):
        cst = dict(zip(const_names, cv))
        idx0, dstw0 = idx[0], dstw[0]
        xc, dinvc, swc = xc[0], dinvc[0], swc[0]
        # conv0: aggregate raw x rows, then embed the aggregate
        aggx = agg64(cst["xtab"], idx0, dstw0, cst["iota16"])   # [64, 8192] f32
        h0T = (xc @ cst["W_e"] + cst["b_e"]).T                   # [128, 8192]
        tx0T = cst["W_e"].T @ aggx + cst["b_e"][:, None] * swc[None, :]
        h1T = jax.nn.relu(cst["W00"].T @ h0T + cst["W01"].T @ tx0T + cst["b0"][:, None])
        u1 = h1T.T.astype(jnp.bfloat16)                          # [8192, 128] node-major
        u1full = jax.lax.all_gather(u1, "x", axis=0, tiled=True)  # [65536, 128]
        agg1 = agg128(u1full, idx0, dstw0, cst["iota16"])        # [128, 8192] f32
        h2T = jax.nn.relu(cst["W10"].T @ h1T + cst["W11"].T @ agg1 + cst["b1"][:, None])
        hm = h2T.reshape(EMB, BPC, E).mean(axis=2)               # [128, 4]
        z = jax.nn.relu(cst["mW1"].T @ hm + cst["mb1"][:, None])
        o = cst["mW2"].T @ z + cst["mb2"][:, None]               # [3, 4]
        return o.T[None]                                         # [1, 4, 3]

    fn = shard_map(
        shard_fn, mesh=mesh,
        in_specs=(P("x"), P("x"), P("x"), P("x"), P("x"))
                 + tuple(P() for _ in const_vals),
        out_specs=P("x"),
        check_rep=False,
    )
    jfn = jax.jit(fn)
    out = jfn(jnp.asarray(idx_all), jnp.asarray(dstw_all),
              jnp.asarray(xT_sh), jnp.asarray(dinv_sh := dinv.reshape(NCORES, NPC)),
              jnp.asarray(sw_sh), *const_vals)
    out1 = np.asarray(out, np.float32).reshape(B, PRED)
    full = np.broadcast_to(out1[:, None, None, :], (B, NPRED, E, PRED))
    return np.ascontiguousarray(full, dtype=np.float32)


kernel._jit_holder = _CACHE

